# revision 58
# baseline (speedup 1.0000x reference)
"""Distributed GQA attention layer (dense_transformer) on 8 TRN2 NeuronCores.

Sharding: 8-way tensor parallel over heads. Core c owns q-heads [4c..4c+4),
kv-head c, and the matching 512 columns/rows of Wq/Wk/Wv/Wo. Each core
computes its heads' attention for both batch rows, the per-core context is
AllGathered (bf16, 4MB/rank), and each core produces a disjoint 512-wide
slice of the output hidden dim via its Wo shard. Host assembles by pure
concatenation.

Layout strategy (per core):
  - hidden^T (bf16, host-pretransposed) streams through SBUF once.
  - QKV projections produce q^T/k^T/v^T [dim, token] directly (weight-
    stationary matmuls, N=512 moving).
  - RoPE applied in [dim, token] layout: partition-swap via a permutation
    matmul on PE, then q*cos + swap*sin on DVE with host-precomputed
    [128, S] tables (sign folded into the sin table, softmax scale folded
    into Wq).
  - Scores are computed transposed: scores^T[s_k, s_q] = k^T.T @ q^T, so
    softmax exp tiles feed PV directly as the moving operand:
    ctx^T[d, s_q] = V[s_k, d].T @ exp[s_k, s_q]. Causal masking = skip
    fully-masked chunks + one triangular 128x128 mask on diagonal blocks.
    The softmax denominator accumulates in its own PSUM bank via
    column-0-of-ones stationary matmuls, batched 4 full-width chunks per
    matmul by pre-summing exp tiles on DVE; per-head normalization is
    reciprocal of den row 0 (DVE) -> partition_broadcast (Pool engine)
    -> multiply (DVE), with the chain emitted mid-next-head so the
    strict-FIFO DVE queue keeps it off the next head's critical path.
    PSUM: 3 score banks (the 3-deep pipeline takes the exp WAR off the
    score-issue path) + 2 ctx + 1 den + 2 o_proj accumulators = 8.
  - o_proj contracts over the gathered [4096, token] context with the Wo
    shard SBUF-resident.
  - The AllGather is split into 8 per-token-block gathers (512KB/rank each)
    and block o_proj is emitted after the NEXT block's attention, so both
    the collectives and the gathered-context DMAs hide under attention
    compute on the in-order PE stream.
"""
import sys
sys.path.insert(0, "/opt/trn_rl_repo")

import numpy as np
import ml_dtypes

import concourse.bass as bass
import concourse.tile as tile
from concourse import bacc, mybir

BF16 = mybir.dt.bfloat16
F32 = mybir.dt.float32
NPBF16 = ml_dtypes.bfloat16

N_CORES = 8
B, S, HID = 2, 2048, 4096
NH, KVH, D = 32, 8, 128
TOK = B * S                # 4096 tokens, batch-major
QO = NH * D // N_CORES     # 512 q-out dims per core
TT = 512                   # token tile (moving free dim)
NTT = TOK // TT            # 8 token tiles
KC = HID // 128            # 32 contraction chunks


def _build(sim=False, loop_k=1, simpden=False, ablate=(), pipe3=False):
    # sim=True: single-core variant for TimelineSim (cost-model timing) —
    # the AllGather is replaced by a local DMA of this core's slice.
    # loop_k>1: timing variant — each compute phase repeats loop_k times
    # inside a hardware For_i loop so device time dominates dispatch noise.
    # ablate: timing-only build variants with parts removed (WRONG OUTPUT):
    #   "noden" (no softmax denominator), "nooproj" (no o_proj),
    #   "nogather" (no ctx DMA round-trip; o_proj reads static SBUF),
    #   "nohdma" (h tiles loaded once, not per-tile), "norope" (plain copy).
    ablate = set(ablate)
    nc = bacc.Bacc("TRN2", target_bir_lowering=False, debug=False,
                   num_devices=1 if sim else N_CORES)
    import contextlib

    if isinstance(loop_k, int):
        loop_k = (loop_k, loop_k, loop_k)
    _phase_idx = iter([0, 1, 2])

    def phase_loop(tc):
        k = loop_k[next(_phase_idx)]
        if k > 1:
            return tc.For_i(0, k, 1)
        return contextlib.nullcontext()
    hid_t = nc.dram_tensor("hid_t", [HID, TOK], BF16, kind="ExternalInput").ap()
    wq_t = nc.dram_tensor("wq_t", [HID, QO], BF16, kind="ExternalInput").ap()
    wk_t = nc.dram_tensor("wk_t", [HID, D], BF16, kind="ExternalInput").ap()
    wv_t = nc.dram_tensor("wv_t", [HID, D], BF16, kind="ExternalInput").ap()
    wo_t = nc.dram_tensor("wo_t", [HID, QO], BF16, kind="ExternalInput").ap()
    cos_t = nc.dram_tensor("cos_t", [D, S], F32, kind="ExternalInput").ap()
    sin_t = nc.dram_tensor("sin_t", [D, S], F32, kind="ExternalInput").ap()
    perm_d = nc.dram_tensor("perm", [128, 128], BF16, kind="ExternalInput").ap()
    ident_d = nc.dram_tensor("ident", [128, 128], BF16, kind="ExternalInput").ap()
    tri_d = nc.dram_tensor("tri", [128, 128], BF16, kind="ExternalInput").ap()
    ones4_d = nc.dram_tensor("ones4", [128, 128], BF16, kind="ExternalInput").ap()
    out = nc.dram_tensor("out", [TOK, QO], F32, kind="ExternalOutput").ap()

    EXP = mybir.ActivationFunctionType.Exp

    with tile.TileContext(nc) as tc:
        with tc.tile_pool(name="const", bufs=1) as cst, \
             tc.tile_pool(name="persist", bufs=1) as per, \
             tc.tile_pool(name="dram", bufs=1, space="DRAM") as dram:
            # cos/sin ride the scalar queue so the sync queue's head stays
            # clear for the first weight/h chunks the PE is gated on
            cos_sb = cst.tile([D, S], F32)
            nc.scalar.dma_start(out=cos_sb, in_=cos_t)
            sin_sb = cst.tile([D, S], F32)
            nc.scalar.dma_start(out=sin_sb, in_=sin_t)
            perm_sb = cst.tile([128, 128], BF16)
            nc.sync.dma_start(out=perm_sb, in_=perm_d)
            ident_sb = cst.tile([128, 128], BF16)
            nc.sync.dma_start(out=ident_sb, in_=ident_d)
            tri_sb = cst.tile([128, 128], BF16)
            nc.sync.dma_start(out=tri_sb, in_=tri_d)
            ones_sb = cst.tile([128, 1], BF16)
            nc.vector.memset(ones_sb, 1.0)
            ones4_sb = cst.tile([128, 128], BF16)
            nc.sync.dma_start(out=ones4_sb, in_=ones4_d)
            # column-0-of-ones: den accumulator lhsT with full 128x128
            # tile_size so the PE never reconfigures between pv and den
            dcol_sb = cst.tile([128, 128], BF16)
            nc.vector.memset(dcol_sb, 0.0)
            nc.vector.memset(dcol_sb[:, 0:1], 1.0)
            ex_const = None
            if "noexp" in ablate:
                ex_const = cst.tile([128, TT], BF16)
                nc.vector.memset(ex_const, 0.01)

            q_rope = per.tile([128, 4, TOK], BF16)    # [d, head, token]
            k_rope = per.tile([128, TOK], BF16)       # [d, token]
            v_sb = per.tile([128, KC, 128], BF16)     # [tok%128, tokchunk, d]

            cc_in = [dram.tile([QO, TT], BF16, name=f"ccin{i}")
                     for i in range(NTT)]
            cc_out = [dram.tile([N_CORES * QO, TT], BF16, addr_space="Shared",
                                name=f"ccout{i}")
                      for i in range(NTT)]

            # ---------------- QKV projections + RoPE ----------------
            with tc.tile_pool(name="wqkv", bufs=1) as wp, \
                 tc.tile_pool(name="hin", bufs=2) as hp, \
                 tc.tile_pool(name="qk_ps", bufs=1, space="PSUM") as aps, \
                 tc.tile_pool(name="rope_ps", bufs=1, space="PSUM") as rps, \
                 tc.tile_pool(name="ropesb", bufs=2) as rsb:
                wq_sb = wp.tile([128, KC, QO], BF16)
                wq_r = wq_t.rearrange("(c p) m -> p c m", p=128)
                for q4 in range(4):
                    eng = nc.sync if q4 % 2 == 0 else nc.scalar
                    eng.dma_start(out=wq_sb[:, q4 * 8:(q4 + 1) * 8, :],
                                  in_=wq_r[:, q4 * 8:(q4 + 1) * 8, :])
                wk_sb = wp.tile([128, KC, D], BF16)
                nc.sync.dma_start(out=wk_sb,
                                  in_=wk_t.rearrange("(c p) m -> p c m", p=128))
                wv_sb = wp.tile([128, KC, D], BF16)
                nc.scalar.dma_start(out=wv_sb,
                                    in_=wv_t.rearrange("(c p) m -> p c m", p=128))

                hid_r = hid_t.rearrange("(c p) t -> p c t", p=128)
                h_static = None
                if "nohdma" in ablate:
                    h_static = wp.tile([128, KC, TT], BF16)
                    nc.sync.dma_start(out=h_static,
                                      in_=hid_r[:, :, 0:TT])
                with phase_loop(tc):
                  for tt in range(NTT):
                      pos0 = (tt % (S // TT)) * TT
                      if h_static is not None:
                          h_q = [h_static[:, q4 * 8:(q4 + 1) * 8, :]
                                 for q4 in range(4)]
                      else:
                        # quarter tiles: the first matmul row gates on 1MB
                        # of h, not the full 4MB tile
                        h_q = []
                        for q4 in range(4):
                          hq = hp.tile([128, 8, TT], BF16, tag=f"h{q4}",
                                       name=f"h{q4}")
                          eng = nc.sync if q4 % 2 == 0 else nc.gpsimd
                          eng.dma_start(
                              out=hq,
                              in_=hid_r[:, q4 * 8:(q4 + 1) * 8,
                                        tt * TT:(tt + 1) * TT])
                          h_q.append(hq)

                      accs = [aps.tile([128, TT], F32, tag=f"acc{i}",
                                       name=f"acc{i}")
                              for i in range(6)]
                      cs = cos_sb[:, pos0:pos0 + TT]
                      ss = sin_sb[:, pos0:pos0 + TT]

                      def emit_drain(m):
                          # RoPE (q/k) or cast+transpose (v) of a finished
                          # accumulator; emitted one row later so the PE
                          # perm/transpose never waits on the Act copy.
                          acc = accs[m]
                          if m == 5:
                              vbf = rsb.tile([128, TT], BF16, tag="vbf")
                              nc.scalar.copy(vbf, acc)
                              for j in range(4):
                                  vtp = rps.tile([128, 128], BF16, tag="vtp")
                                  nc.tensor.transpose(
                                      vtp, vbf[:, j * 128:(j + 1) * 128],
                                      ident_sb)
                                  nc.vector.tensor_copy(
                                      v_sb[:, tt * 4 + j, :], vtp)
                              return
                          dest = (q_rope[:, m, tt * TT:(tt + 1) * TT]
                                  if m < 4 else
                                  k_rope[:, tt * TT:(tt + 1) * TT])
                          if "norope" in ablate:
                              nc.vector.tensor_copy(dest, acc)
                              return
                          xbf = rsb.tile([128, TT], BF16, tag="xbf")
                          nc.scalar.copy(xbf, acc)
                          swp = rps.tile([128, TT], F32, tag="swp")
                          nc.tensor.matmul(swp, lhsT=perm_sb, rhs=xbf,
                                           start=True, stop=True)
                          t2 = rsb.tile([128, TT], F32, tag="t2")
                          nc.vector.tensor_mul(t2, swp, ss)
                          t1 = rsb.tile([128, TT], F32, tag="t1")
                          nc.vector.tensor_mul(t1, acc, cs)
                          nc.vector.tensor_add(dest, t1, t2)

                      # m-outer: each accumulator gets 32 consecutive
                      # same-bank matmuls (measured ~15% cheaper per matmul
                      # than bank-alternating), and each finished row's
                      # RoPE drain hides under the next row's matmuls.
                      row_lhsT = (
                          [lambda kc, m=m: wq_sb[:, kc, m * 128:(m + 1) * 128]
                           for m in range(4)]
                          + [lambda kc: wk_sb[:, kc, :],
                             lambda kc: wv_sb[:, kc, :]])
                      for m in range(6):
                          for kc in range(KC):
                              nc.tensor.matmul(
                                  accs[m], lhsT=row_lhsT[m](kc),
                                  rhs=h_q[kc // 8][:, kc % 8, :],
                                  start=kc == 0, stop=kc == KC - 1)
                          if m >= 1:
                              emit_drain(m - 1)
                      emit_drain(5)

            # ------------- attention / AllGather / o_proj (fused) -------------
            # Block pipeline over NTT token blocks of 512: attention for the
            # block's 4 heads -> per-block AllGather -> block o_proj, with
            # o_proj(blk) emitted after attention(blk+1) so the PE never
            # waits on a collective in flight.
            def emit_attn_tile(cps, asb, b, h, t, cc_dst, sc_bufs,
                               ctx_bufs=2, den_bufs=2, insert_cb=None):
                tok0 = b * S + t * TT
                nkc = 4 * t + 4
                ctx = cps.tile([128, TT], F32, tag="ctx", bufs=ctx_bufs,
                               name="ctx")
                den = None
                if "noden" not in ablate:
                    # row 0 accumulates the softmax denominator; rows 1..127
                    # accumulate zeros (dcol_sb columns 1..127 are zero).
                    # chunk 0 always spans the full [0:TT] (a0=0), so the
                    # first den matmul's start=True initializes every column.
                    den = cps.tile([128, TT], F32, tag="den", bufs=den_bufs,
                                   name="den")

                def a0_of(kc):
                    return max(kc * 128 - t * TT, 0)

                # software pipeline: PE stream is score(k+2), pv(k). The
                # denominator batches up to 4 full-width exp tiles into one
                # matmul: DVE (idle here) pre-sums them pairwise, so den
                # costs ~1/4 of the per-instruction PE overhead. Partial
                # (diagonal-region) chunks get individual den matmuls; the
                # last chunk is always partial, so it carries stop=True.
                scs, exs = {}, {}
                n_full = 4 * t + 1
                den_started = [False]
                quad = [None, 0]   # running quad sum tile, member count
                dq = []            # completed quad sums: (last_j, tile)

                def den_mm(rhs_ap, a0d, sp):
                    nc.tensor.matmul(den[:, a0d:], lhsT=dcol_sb, rhs=rhs_ap,
                                     start=not den_started[0], stop=sp)
                    den_started[0] = True

                ins_at = 5 if nkc > 5 else 2
                for kc in range(nkc + 2):
                    if kc == ins_at and insert_cb is not None:
                        # previous head's deferred normalize chain: emitted
                        # mid-head so its DVE ops queue BEHIND this head's
                        # first quad-adds (strict-FIFO DVE) — the first den
                        # matmul (in-order PE) then never waits on it
                        insert_cb()
                    if kc < nkc:
                        a0 = a0_of(kc)
                        sc = cps.tile([128, TT], F32, tag="sc",
                                      bufs=sc_bufs, name="sc")
                        nc.tensor.matmul(
                            sc[:, a0:],
                            lhsT=k_rope[:, b * S + kc * 128:
                                        b * S + (kc + 1) * 128],
                            rhs=q_rope[:, h, tok0 + a0:tok0 + TT],
                            start=True, stop=True)
                        scs[kc] = sc
                    if 1 <= kc <= nkc:
                        j = kc - 1
                        a0 = a0_of(j)
                        sc = scs.pop(j)
                        if "noexp" in ablate:
                            exs[j] = ex_const
                        else:
                            ex = asb.tile([128, TT], BF16, tag="ex",
                                          bufs=8, name="ex")
                            nc.scalar.activation(ex[:, a0:], sc[:, a0:], EXP)
                            if a0 == j * 128 - t * TT:
                                # diagonal block: triangular mask
                                nc.vector.tensor_mul(ex[:, a0:a0 + 128],
                                                     ex[:, a0:a0 + 128],
                                                     tri_sb)
                            exs[j] = ex
                        ex = exs[j]
                        # den batching: chain full-width exp tiles into a
                        # quad sum on DVE as each exp lands, so the batched
                        # den matmul (1 per 4 chunks) never waits on DVE
                        if ("noden" not in ablate and not simpden
                                and j < n_full):
                            if quad[1] == 0:
                                quad[0] = ex
                            elif quad[1] == 1:
                                s = asb.tile([128, TT], BF16, tag="exsum",
                                             bufs=2, name="exsum")
                                nc.vector.tensor_add(s, quad[0], ex)
                                quad[0] = s
                            else:
                                nc.vector.tensor_add(quad[0], quad[0], ex)
                            quad[1] += 1
                            if quad[1] == 4 or j == n_full - 1:
                                dq.append((j, quad[0]))
                                quad[0], quad[1] = None, 0
                    if kc >= 2:
                        j = kc - 2
                        a0 = a0_of(j)
                        st, sp = j == 0, j == nkc - 1
                        nc.tensor.matmul(ctx[:, a0:],
                                         lhsT=v_sb[:, b * 16 + j, :],
                                         rhs=exs[j][:, a0:],
                                         start=st, stop=sp)
                        if "noden" in ablate:
                            exs.pop(j)
                        elif simpden:
                            nc.tensor.matmul(den[0:1, a0:], lhsT=ones_sb,
                                             rhs=exs.pop(j)[:, a0:],
                                             start=st, stop=sp)
                        elif j < n_full:
                            exs.pop(j)
                            while dq and dq[0][0] < j:
                                den_mm(dq.pop(0)[1][:, :], 0, False)
                        else:
                            while dq:
                                den_mm(dq.pop(0)[1][:, :], 0, False)
                            den_mm(exs.pop(j)[:, a0:], a0, sp)
                def finish():
                    if "nonorm" in ablate or "noden" in ablate:
                        ctxn = asb.tile([128, TT], BF16, tag="ctxn")
                        nc.vector.tensor_copy(ctxn, ctx)
                    else:
                        # normalize: 1/den from PSUM row 0 on DVE, then
                        # partition-broadcast on the idle Pool engine,
                        # multiply back on DVE
                        rd1 = asb.tile([1, TT], F32, tag="rd1")
                        nc.vector.reciprocal(rd1, den[0:1, :])
                        rden = asb.tile([128, TT], F32, tag="rden")
                        nc.gpsimd.partition_broadcast(rden, rd1)
                        ctxn = asb.tile([128, TT], BF16, tag="ctxn")
                        nc.vector.tensor_mul(ctxn, ctx, rden)
                    if "nogather" not in ablate:
                        nc.sync.dma_start(out=cc_dst, in_=ctxn)
                return finish

            def emit_oproj_mg(ops, osb, wo_sb, mg, src_r, src_c0,
                              g_static=None):
                # two passes of 2 output m-tiles each: same matmul count,
                # half the PSUM banks (leaves room for the attention's
                # denominator machinery)
                if g_static is not None:
                    g = g_static
                else:
                    g = osb.tile([128, KC, TT], BF16, tag="g", bufs=2)
                    # split the 4MB load across both HWDGE rings
                    for q4 in range(4):
                        eng = nc.sync if q4 % 2 == 0 else nc.scalar
                        eng.dma_start(
                            out=g[:, q4 * 8:(q4 + 1) * 8, :],
                            in_=src_r[:, q4 * 8:(q4 + 1) * 8,
                                      src_c0:src_c0 + TT])
                for m in range(4):
                    # 32 consecutive same-bank accumulations per output
                    # tile; bufs=2 suffices (drain copy is ~0.4us vs 6.8us
                    # of accumulation) and frees a PSUM bank for den
                    omt = ops.tile([128, QO], F32, tag="om", bufs=2,
                                   name="omt")
                    for kc in range(KC):
                        nc.tensor.matmul(
                            omt,
                            lhsT=g[:, kc, m * 128:(m + 1) * 128],
                            rhs=wo_sb[:, kc, :],
                            start=kc == 0, stop=kc == KC - 1)
                    ofin = osb.tile([128, QO], F32, tag="ofin", bufs=2)
                    # drain on the Activation engine (idle during o_proj),
                    # keeping DVE free for the attention-side work
                    nc.scalar.copy(ofin, omt)
                    nc.gpsimd.dma_start(
                        out=out[mg * TT + m * 128:
                                mg * TT + (m + 1) * 128, :],
                        in_=ofin)

            no_collective = sim or (loop_k[1] > 1)
            with tc.tile_pool(name="fu_ps", bufs=1, space="PSUM") as cps, \
                 tc.tile_pool(name="at_sb", bufs=2) as asb, \
                 tc.tile_pool(name="wo", bufs=1) as wop, \
                 tc.tile_pool(name="o_ps", bufs=1, space="PSUM") as ops, \
                 tc.tile_pool(name="o_sb", bufs=3) as osb:
                wo_sb = wop.tile([128, KC, QO], BF16)
                nc.sync.dma_start(out=wo_sb,
                                  in_=wo_t.rearrange("(c p) m -> p c m", p=128))
                g_static = None
                if "nogather" in ablate:
                    g_static = wop.tile([128, KC, TT], BF16)
                    nc.vector.memset(g_static, 0.001)
                blk_out_r = [
                    cc_out[blk][:].rearrange("(c p) t -> p c t", p=128)
                    for blk in range(NTT)
                ]
                with phase_loop(tc):
                  for blk in range(NTT + 1):
                    if blk < NTT:
                        b, t = divmod(blk, S // TT)
                        pending = None
                        for h in range(4):
                            fin = emit_attn_tile(
                                cps, asb, b, h, t,
                                cc_in[blk][h * 128:(h + 1) * 128, :],
                                sc_bufs=3, ctx_bufs=2, den_bufs=1,
                                insert_cb=pending)
                            pending = fin
                        # head 3's chain must precede the gather emission
                        # (the gather's dependency on cc writes is tracked
                        # by emission order)
                        pending()
                        if "nogather" in ablate:
                            pass
                        elif no_collective:
                            nc.sync.dma_start(out=cc_out[blk][0:QO, :],
                                              in_=cc_in[blk][:])
                        else:
                            nc.gpsimd.collective_compute(
                                "AllGather", mybir.AluOpType.bypass,
                                replica_groups=[list(range(N_CORES))],
                                ins=[cc_in[blk][:].opt()],
                                outs=[cc_out[blk][:].opt()])
                    if blk >= 1 and "nooproj" not in ablate:
                        emit_oproj_mg(ops, osb, wo_sb, blk - 1,
                                      blk_out_r[blk - 1], 0,
                                      g_static=g_static)
                if loop_k[2] > 1:
                    # empty third phase: measures pure For_i overhead
                    # (all-engine barrier + back edge) per iteration
                    dummy = osb.tile([128, 1], F32, tag="dummy")
                    with phase_loop(tc):
                        nc.vector.memset(dummy, 0.0)
    nc.compile()
    return nc


_NC_CACHE = None


def _get_nc():
    global _NC_CACHE
    if _NC_CACHE is None:
        _NC_CACHE = _build()
    return _NC_CACHE


def make_in_maps(hidden_states, position_ids, Wq, Wk, Wv, Wo):
    hs = np.ascontiguousarray(
        np.asarray(hidden_states, dtype=np.float32).reshape(TOK, HID).T
    ).astype(NPBF16)
    pos = np.asarray(position_ids, dtype=np.float32)
    inv = 1.0 / (10000.0 ** (np.arange(0, D, 2, dtype=np.float32) / D))
    fr = pos[:, None] * inv[None, :]                     # [S, 64]
    emb = np.concatenate([fr, fr], axis=-1)              # [S, D]
    cos = np.cos(emb).T.astype(np.float32)               # [D, S]
    sin = np.sin(emb).T.astype(np.float32)
    sin[:64] *= -1.0                                     # fold rotate-half sign
    perm = np.zeros((128, 128), np.float32)
    perm[np.arange(128), (np.arange(128) + 64) % 128] = 1.0
    ident = np.eye(128, dtype=np.float32)
    tri = (np.arange(128)[:, None] <= np.arange(128)[None, :]).astype(np.float32)
    ones4 = np.zeros((128, 128), np.float32)
    ones4[[0, 32, 64, 96], :] = 1.0

    scale = 1.0 / np.sqrt(D)
    Wq = np.asarray(Wq, dtype=np.float32)
    Wk = np.asarray(Wk, dtype=np.float32)
    Wv = np.asarray(Wv, dtype=np.float32)
    Wo = np.asarray(Wo, dtype=np.float32)

    in_maps = []
    for c in range(N_CORES):
        in_maps.append({
            "hid_t": hs,
            "wq_t": np.ascontiguousarray(
                (Wq[c * QO:(c + 1) * QO] * scale).T).astype(NPBF16),
            "wk_t": np.ascontiguousarray(Wk[c * D:(c + 1) * D].T).astype(NPBF16),
            "wv_t": np.ascontiguousarray(Wv[c * D:(c + 1) * D].T).astype(NPBF16),
            "wo_t": np.ascontiguousarray(Wo[c * QO:(c + 1) * QO].T).astype(NPBF16),
            "cos_t": cos,
            "sin_t": sin,
            "perm": perm.astype(NPBF16),
            "ident": ident.astype(NPBF16),
            "tri": tri.astype(NPBF16),
            "ones4": ones4.astype(NPBF16),
        })
    return in_maps


def assemble(results):
    full = np.empty((TOK, HID), np.float32)
    for c in range(N_CORES):
        full[:, c * QO:(c + 1) * QO] = results[c]["out"]
    return full.reshape(B, S, HID)


_RUNNER_CACHE = None


def _make_runner(nc):
    """Build the sharded PJRT callable once so repeat kernel() calls skip
    re-tracing; mirrors concourse.bass2jax.run_bass_via_pjrt."""
    import jax
    from jax.sharding import Mesh, PartitionSpec, NamedSharding
    from jax.experimental.shard_map import shard_map
    from concourse import bass2jax

    bass2jax.install_neuronx_cc_hook()
    partition_name = nc.partition_id_tensor.name if nc.partition_id_tensor else None
    in_names, out_names, out_avals = [], [], []
    for alloc in nc.m.functions[0].allocations:
        if not isinstance(alloc, mybir.MemoryLocationSet):
            continue
        name = alloc.memorylocations[0].name
        if alloc.kind == "ExternalInput":
            if name != partition_name:
                in_names.append(name)
        elif alloc.kind == "ExternalOutput":
            out_names.append(name)
            out_avals.append(jax.core.ShapedArray(
                tuple(alloc.tensor_shape), mybir.dt.np(alloc.dtype)))
    n_params, n_outs = len(in_names), len(out_avals)

    def _body(*args):
        operands = list(args)
        if partition_name is not None:
            operands.append(bass2jax.partition_id_tensor())
        return tuple(bass2jax._bass_exec_p.bind(
            *operands,
            out_avals=tuple(out_avals),
            in_names=tuple(in_names + out_names
                           + ([partition_name] if partition_name else [])),
            out_names=tuple(out_names),
            lowering_input_output_aliases=(),
            sim_require_finite=True,
            sim_require_nnan=True,
            nc=nc,
        ))

    devices = jax.devices()[:N_CORES]
    mesh = Mesh(np.asarray(devices), ("core",))
    fn = jax.jit(
        shard_map(_body, mesh=mesh,
                  in_specs=(PartitionSpec("core"),) * (n_params + n_outs),
                  out_specs=(PartitionSpec("core"),) * n_outs,
                  check_rep=False),
        keep_unused=True,
    )
    sharding = NamedSharding(mesh, PartitionSpec("core"))

    def run(in_maps):
        per_core = [[np.asarray(m[name]) for name in in_names] for m in in_maps]
        concat_in = [
            np.concatenate([per_core[c][i] for c in range(N_CORES)], axis=0)
            for i in range(n_params)
        ]
        concat_zeros = [
            np.zeros((N_CORES * a.shape[0], *a.shape[1:]), a.dtype)
            for a in out_avals
        ]
        import jax as _jax
        dev_args = [_jax.device_put(a, sharding)
                    for a in concat_in + concat_zeros]
        outs = fn(*dev_args)
        _jax.block_until_ready(outs)
        return [
            {name: np.asarray(outs[i]).reshape(N_CORES, *out_avals[i].shape)[c]
             for i, name in enumerate(out_names)}
            for c in range(N_CORES)
        ]

    return run


def kernel(hidden_states, position_ids, Wq, Wk, Wv, Wo):
    global _RUNNER_CACHE
    nc = _get_nc()
    in_maps = make_in_maps(hidden_states, position_ids, Wq, Wk, Wv, Wo)
    try:
        if _RUNNER_CACHE is None:
            _RUNNER_CACHE = _make_runner(nc)
        return assemble(_RUNNER_CACHE(in_maps))
    except Exception:
        from concourse.bass_utils import run_bass_kernel_spmd
        res = run_bass_kernel_spmd(nc, in_maps, core_ids=list(range(N_CORES)))
        return assemble(res.results)



# revision 66
# speedup vs baseline: 1.0547x; 1.0547x over previous
"""Distributed GQA attention layer (dense_transformer) on 8 TRN2 NeuronCores.

Sharding: 8-way tensor parallel over heads. Core c owns q-heads [4c..4c+4),
kv-head c, and the matching 512 columns/rows of Wq/Wk/Wv/Wo. Each core
computes its heads' attention for both batch rows, the per-core context is
AllGathered (bf16, 4MB/rank), and each core produces a disjoint 512-wide
slice of the output hidden dim via its Wo shard. Host assembles by pure
concatenation.

Layout strategy (per core):
  - hidden^T (bf16, host-pretransposed) streams through SBUF once.
  - QKV projections produce q^T/k^T/v^T [dim, token] directly (weight-
    stationary matmuls, N=512 moving).
  - RoPE applied in [dim, token] layout: partition-swap via a permutation
    matmul on PE, then q*cos + swap*sin on DVE with host-precomputed
    [128, S] tables (sign folded into the sin table, softmax scale folded
    into Wq).
  - Scores are computed transposed: scores^T[s_k, s_q] = k^T.T @ q^T, so
    softmax exp tiles feed PV directly as the moving operand:
    ctx^T[d, s_q] = V[s_k, d].T @ exp[s_k, s_q]. Causal masking = skip
    fully-masked chunks + one triangular 128x128 mask on diagonal blocks.
    The softmax denominator accumulates in its own PSUM bank via
    column-0-of-ones stationary matmuls, batched 4 full-width chunks per
    matmul by pre-summing exp tiles on DVE; per-head normalization is
    reciprocal of den row 0 (DVE) -> partition_broadcast (Pool engine)
    -> multiply (DVE), with the chain emitted mid-next-head so the
    strict-FIFO DVE queue keeps it off the next head's critical path.
    PSUM: 3 score banks (the 3-deep pipeline takes the exp WAR off the
    score-issue path) + 2 ctx + 1 den + 2 o_proj accumulators = 8.
  - o_proj contracts over the gathered [4096, token] context with the Wo
    shard SBUF-resident.
  - The AllGather is split into 8 per-token-block gathers (512KB/rank each)
    and block o_proj is emitted after the NEXT block's attention, so both
    the collectives and the gathered-context DMAs hide under attention
    compute on the in-order PE stream.
"""
import sys
sys.path.insert(0, "/opt/trn_rl_repo")

import numpy as np
import ml_dtypes

import concourse.bass as bass
import concourse.tile as tile
from concourse import bacc, mybir

BF16 = mybir.dt.bfloat16
F32 = mybir.dt.float32
NPBF16 = ml_dtypes.bfloat16

N_CORES = 8
B, S, HID = 2, 2048, 4096
NH, KVH, D = 32, 8, 128
TOK = B * S                # 4096 tokens, batch-major
QO = NH * D // N_CORES     # 512 q-out dims per core
TT = 512                   # token tile (moving free dim)
NTT = TOK // TT            # 8 token tiles
KC = HID // 128            # 32 contraction chunks


def _build(sim=False, loop_k=1, simpden=False, ablate=(), pipe3=False,
           denq=4):
    # sim=True: single-core variant for TimelineSim (cost-model timing) —
    # the AllGather is replaced by a local DMA of this core's slice.
    # loop_k>1: timing variant — each compute phase repeats loop_k times
    # inside a hardware For_i loop so device time dominates dispatch noise.
    # ablate: timing-only build variants with parts removed (WRONG OUTPUT):
    #   "noden" (no softmax denominator), "nooproj" (no o_proj),
    #   "nogather" (no ctx DMA round-trip; o_proj reads static SBUF),
    #   "nohdma" (h tiles loaded once, not per-tile), "norope" (plain copy).
    ablate = set(ablate)
    nc = bacc.Bacc("TRN2", target_bir_lowering=False, debug=False,
                   num_devices=1 if sim else N_CORES)
    import contextlib

    if isinstance(loop_k, int):
        loop_k = (loop_k, loop_k, loop_k)
    _phase_idx = iter([0, 1, 2])

    def phase_loop(tc):
        k = loop_k[next(_phase_idx)]
        if k > 1:
            return tc.For_i(0, k, 1)
        return contextlib.nullcontext()
    hid_t = nc.dram_tensor("hid_t", [HID, TOK], BF16, kind="ExternalInput").ap()
    wq_t = nc.dram_tensor("wq_t", [HID, QO], BF16, kind="ExternalInput").ap()
    wk_t = nc.dram_tensor("wk_t", [HID, D], BF16, kind="ExternalInput").ap()
    wv_t = nc.dram_tensor("wv_t", [HID, D], BF16, kind="ExternalInput").ap()
    wo_t = nc.dram_tensor("wo_t", [HID, QO], BF16, kind="ExternalInput").ap()
    cos_t = nc.dram_tensor("cos_t", [D, S], F32, kind="ExternalInput").ap()
    sin_t = nc.dram_tensor("sin_t", [D, S], F32, kind="ExternalInput").ap()
    perm_d = nc.dram_tensor("perm", [128, 128], BF16, kind="ExternalInput").ap()
    ident_d = nc.dram_tensor("ident", [128, 128], BF16, kind="ExternalInput").ap()
    tri_d = nc.dram_tensor("tri", [128, 128], BF16, kind="ExternalInput").ap()
    ones4_d = nc.dram_tensor("ones4", [128, 128], BF16, kind="ExternalInput").ap()
    out = nc.dram_tensor("out", [TOK, QO], F32, kind="ExternalOutput").ap()

    EXP = mybir.ActivationFunctionType.Exp

    with tile.TileContext(nc) as tc:
        with tc.tile_pool(name="const", bufs=1) as cst, \
             tc.tile_pool(name="persist", bufs=1) as per, \
             tc.tile_pool(name="dram", bufs=1, space="DRAM") as dram:
            # cos/sin ride the scalar queue so the sync queue's head stays
            # clear for the first weight/h chunks the PE is gated on
            cos_sb = cst.tile([D, S], F32)
            nc.scalar.dma_start(out=cos_sb, in_=cos_t)
            sin_sb = cst.tile([D, S], F32)
            nc.scalar.dma_start(out=sin_sb, in_=sin_t)
            perm_sb = cst.tile([128, 128], BF16)
            nc.sync.dma_start(out=perm_sb, in_=perm_d)
            ident_sb = cst.tile([128, 128], BF16)
            nc.sync.dma_start(out=ident_sb, in_=ident_d)
            tri_sb = cst.tile([128, 128], BF16)
            nc.sync.dma_start(out=tri_sb, in_=tri_d)
            ones_sb = cst.tile([128, 1], BF16)
            nc.vector.memset(ones_sb, 1.0)
            ones4_sb = cst.tile([128, 128], BF16)
            nc.sync.dma_start(out=ones4_sb, in_=ones4_d)
            # column-0-of-ones: den accumulator lhsT with full 128x128
            # tile_size so the PE never reconfigures between pv and den
            dcol_sb = cst.tile([128, 128], BF16)
            nc.vector.memset(dcol_sb, 0.0)
            nc.vector.memset(dcol_sb[:, 0:1], 1.0)
            # additive causal mask (0 where k<=q, -30000 where k>q) applied
            # by a PE matmul accumulated onto diagonal score blocks
            mtri_sb = cst.tile([128, 128], BF16)
            nc.scalar.activation(mtri_sb, tri_sb,
                                 mybir.ActivationFunctionType.Copy,
                                 bias=-30000.0, scale=30000.0)
            ex_const = None
            if "noexp" in ablate:
                ex_const = cst.tile([128, TT], BF16)
                nc.vector.memset(ex_const, 0.01)

            q_rope = per.tile([128, 4, TOK], BF16)    # [d, head, token]
            k_rope = per.tile([128, TOK], BF16)       # [d, token]
            v_sb = per.tile([128, KC, 128], BF16)     # [tok%128, tokchunk, d]

            cc_in = [dram.tile([QO, TT], BF16, name=f"ccin{i}")
                     for i in range(NTT)]
            cc_out = [dram.tile([N_CORES * QO, TT], BF16, addr_space="Shared",
                                name=f"ccout{i}")
                      for i in range(NTT)]

            # ---------------- QKV projections + RoPE ----------------
            with tc.tile_pool(name="wqkv", bufs=1) as wp, \
                 tc.tile_pool(name="hin", bufs=2) as hp, \
                 tc.tile_pool(name="qk_ps", bufs=1, space="PSUM") as aps, \
                 tc.tile_pool(name="rope_ps", bufs=1, space="PSUM") as rps, \
                 tc.tile_pool(name="ropesb", bufs=2) as rsb:
                wq_sb = wp.tile([128, KC, QO], BF16)
                wq_r = wq_t.rearrange("(c p) m -> p c m", p=128)
                for q4 in range(4):
                    eng = nc.sync if q4 % 2 == 0 else nc.scalar
                    eng.dma_start(out=wq_sb[:, q4 * 8:(q4 + 1) * 8, :],
                                  in_=wq_r[:, q4 * 8:(q4 + 1) * 8, :])
                wk_sb = wp.tile([128, KC, D], BF16)
                nc.sync.dma_start(out=wk_sb,
                                  in_=wk_t.rearrange("(c p) m -> p c m", p=128))
                wv_sb = wp.tile([128, KC, D], BF16)
                nc.scalar.dma_start(out=wv_sb,
                                    in_=wv_t.rearrange("(c p) m -> p c m", p=128))

                hid_r = hid_t.rearrange("(c p) t -> p c t", p=128)
                h_static = None
                if "nohdma" in ablate:
                    h_static = wp.tile([128, KC, TT], BF16)
                    nc.sync.dma_start(out=h_static,
                                      in_=hid_r[:, :, 0:TT])
                with phase_loop(tc):
                  for tt in range(NTT):
                      pos0 = (tt % (S // TT)) * TT
                      if h_static is not None:
                          h_q = [h_static[:, q4 * 8:(q4 + 1) * 8, :]
                                 for q4 in range(4)]
                      else:
                        # quarter tiles: the first matmul row gates on 1MB
                        # of h, not the full 4MB tile
                        h_q = []
                        for q4 in range(4):
                          hq = hp.tile([128, 8, TT], BF16, tag=f"h{q4}",
                                       name=f"h{q4}")
                          eng = nc.sync if q4 % 2 == 0 else nc.gpsimd
                          eng.dma_start(
                              out=hq,
                              in_=hid_r[:, q4 * 8:(q4 + 1) * 8,
                                        tt * TT:(tt + 1) * TT])
                          h_q.append(hq)

                      accs = [aps.tile([128, TT], F32, tag=f"acc{i}",
                                       name=f"acc{i}")
                              for i in range(6)]
                      cs = cos_sb[:, pos0:pos0 + TT]
                      ss = sin_sb[:, pos0:pos0 + TT]

                      def emit_drain(m):
                          # RoPE (q/k) or cast+transpose (v) of a finished
                          # accumulator; emitted one row later so the PE
                          # perm/transpose never waits on the Act copy.
                          acc = accs[m]
                          if m == 5:
                              vbf = rsb.tile([128, TT], BF16, tag="vbf")
                              nc.scalar.copy(vbf, acc)
                              for j in range(4):
                                  vtp = rps.tile([128, 128], BF16, tag="vtp")
                                  nc.tensor.transpose(
                                      vtp, vbf[:, j * 128:(j + 1) * 128],
                                      ident_sb)
                                  nc.vector.tensor_copy(
                                      v_sb[:, tt * 4 + j, :], vtp)
                              return
                          dest = (q_rope[:, m, tt * TT:(tt + 1) * TT]
                                  if m < 4 else
                                  k_rope[:, tt * TT:(tt + 1) * TT])
                          if "norope" in ablate:
                              nc.vector.tensor_copy(dest, acc)
                              return
                          xbf = rsb.tile([128, TT], BF16, tag="xbf")
                          nc.scalar.copy(xbf, acc)
                          swp = rps.tile([128, TT], F32, tag="swp")
                          nc.tensor.matmul(swp, lhsT=perm_sb, rhs=xbf,
                                           start=True, stop=True)
                          t2 = rsb.tile([128, TT], F32, tag="t2")
                          nc.vector.tensor_mul(t2, swp, ss)
                          t1 = rsb.tile([128, TT], F32, tag="t1")
                          nc.vector.tensor_mul(t1, acc, cs)
                          nc.vector.tensor_add(dest, t1, t2)

                      # m-outer: each accumulator gets 32 consecutive
                      # same-bank matmuls (measured ~15% cheaper per matmul
                      # than bank-alternating), and each finished row's
                      # RoPE drain hides under the next row's matmuls.
                      row_lhsT = (
                          [lambda kc, m=m: wq_sb[:, kc, m * 128:(m + 1) * 128]
                           for m in range(4)]
                          + [lambda kc: wk_sb[:, kc, :],
                             lambda kc: wv_sb[:, kc, :]])
                      for m in range(6):
                          for kc in range(KC):
                              nc.tensor.matmul(
                                  accs[m], lhsT=row_lhsT[m](kc),
                                  rhs=h_q[kc // 8][:, kc % 8, :],
                                  start=kc == 0, stop=kc == KC - 1)
                          if m >= 1:
                              emit_drain(m - 1)
                      emit_drain(5)

            # ------------- attention / AllGather / o_proj (fused) -------------
            # Block pipeline over NTT token blocks of 512: attention for the
            # block's 4 heads -> per-block AllGather -> block o_proj, with
            # o_proj(blk) emitted after attention(blk+1) so the PE never
            # waits on a collective in flight.
            def emit_attn_tile(cps, asb, b, h, t, cc_dst, sc_bufs,
                               ctx_bufs=2, den_bufs=2, insert_cb=None):
                tok0 = b * S + t * TT
                nkc = 4 * t + 4
                ctx = cps.tile([128, TT], F32, tag="ctx", bufs=ctx_bufs,
                               name="ctx")
                den = None
                if "noden" not in ablate:
                    # row 0 accumulates the softmax denominator; rows 1..127
                    # accumulate zeros (dcol_sb columns 1..127 are zero).
                    # chunk 0 always spans the full [0:TT] (a0=0), so the
                    # first den matmul's start=True initializes every column.
                    den = cps.tile([128, TT], F32, tag="den", bufs=den_bufs,
                                   name="den")

                def a0_of(kc):
                    return max(kc * 128 - t * TT, 0)

                # software pipeline: PE stream is score(k+2), pv(k). The
                # denominator batches up to 4 full-width exp tiles into one
                # matmul: DVE (idle here) pre-sums them pairwise, so den
                # costs ~1/4 of the per-instruction PE overhead. Partial
                # (diagonal-region) chunks get individual den matmuls; the
                # last chunk is always partial, so it carries stop=True.
                scs, exs = {}, {}
                n_full = 4 * t + 1
                den_started = [False]
                quad = [None, 0]   # running quad sum tile, member count
                dq = []            # completed quad sums: (last_j, tile)

                def den_mm(rhs_ap, a0d, sp):
                    nc.tensor.matmul(den[:, a0d:], lhsT=dcol_sb, rhs=rhs_ap,
                                     start=not den_started[0], stop=sp)
                    den_started[0] = True

                ins_at = 5 if nkc > 5 else 2
                for kc in range(nkc + 2):
                    if kc == ins_at and insert_cb is not None:
                        # previous head's deferred normalize chain: emitted
                        # mid-head so its DVE ops queue BEHIND this head's
                        # first quad-adds (strict-FIFO DVE) — the first den
                        # matmul (in-order PE) then never waits on it
                        insert_cb()
                    if kc < nkc:
                        a0 = a0_of(kc)
                        diag = a0 == kc * 128 - t * TT
                        sc = cps.tile([128, TT], F32, tag="sc",
                                      bufs=sc_bufs, name="sc")
                        nc.tensor.matmul(
                            sc[:, a0:],
                            lhsT=k_rope[:, b * S + kc * 128:
                                        b * S + (kc + 1) * 128],
                            rhs=q_rope[:, h, tok0 + a0:tok0 + TT],
                            start=True, stop=not diag)
                        if diag:
                            # ident^T @ mtri = mtri, accumulated onto the
                            # diagonal block — replaces a DVE mask multiply
                            nc.tensor.matmul(
                                sc[:, a0:a0 + 128], lhsT=ident_sb,
                                rhs=mtri_sb, start=False, stop=True)
                        scs[kc] = sc
                    if 1 <= kc <= nkc:
                        j = kc - 1
                        a0 = a0_of(j)
                        sc = scs.pop(j)
                        if "noexp" in ablate:
                            exs[j] = ex_const
                        else:
                            ex = asb.tile([128, TT], BF16, tag="ex",
                                          bufs=8, name="ex")
                            nc.scalar.activation(ex[:, a0:], sc[:, a0:], EXP)
                            exs[j] = ex
                        ex = exs[j]
                        # den batching: chain full-width exp tiles into a
                        # quad sum on DVE as each exp lands, so the batched
                        # den matmul (1 per 4 chunks) never waits on DVE
                        if ("noden" not in ablate and not simpden
                                and j < n_full):
                            if quad[1] == 0:
                                quad[0] = ex
                            elif quad[1] == 1:
                                s = asb.tile([128, TT], BF16, tag="exsum",
                                             bufs=2, name="exsum")
                                nc.vector.tensor_add(s, quad[0], ex)
                                quad[0] = s
                            else:
                                nc.vector.tensor_add(quad[0], quad[0], ex)
                            quad[1] += 1
                            if quad[1] == denq or j == n_full - 1:
                                dq.append((j, quad[0]))
                                quad[0], quad[1] = None, 0
                    if kc >= 2:
                        j = kc - 2
                        a0 = a0_of(j)
                        st, sp = j == 0, j == nkc - 1
                        nc.tensor.matmul(ctx[:, a0:],
                                         lhsT=v_sb[:, b * 16 + j, :],
                                         rhs=exs[j][:, a0:],
                                         start=st, stop=sp)
                        if "noden" in ablate:
                            exs.pop(j)
                        elif simpden:
                            nc.tensor.matmul(den[0:1, a0:], lhsT=ones_sb,
                                             rhs=exs.pop(j)[:, a0:],
                                             start=st, stop=sp)
                        elif j < n_full:
                            exs.pop(j)
                            while dq and dq[0][0] < j:
                                den_mm(dq.pop(0)[1][:, :], 0, False)
                        else:
                            while dq:
                                den_mm(dq.pop(0)[1][:, :], 0, False)
                            den_mm(exs.pop(j)[:, a0:], a0, sp)
                def finish():
                    if "nonorm" in ablate or "noden" in ablate:
                        ctxn = asb.tile([128, TT], BF16, tag="ctxn")
                        nc.vector.tensor_copy(ctxn, ctx)
                    else:
                        # normalize: 1/den from PSUM row 0 on DVE, then
                        # partition-broadcast on the idle Pool engine,
                        # multiply back on DVE
                        rd1 = asb.tile([1, TT], F32, tag="rd1")
                        nc.vector.reciprocal(rd1, den[0:1, :])
                        rden = asb.tile([128, TT], F32, tag="rden")
                        nc.gpsimd.partition_broadcast(rden, rd1)
                        ctxn = asb.tile([128, TT], BF16, tag="ctxn")
                        nc.vector.tensor_mul(ctxn, ctx, rden)
                    if "nogather" not in ablate:
                        nc.sync.dma_start(out=cc_dst, in_=ctxn)
                return finish

            def emit_oproj_mg(ops, osb, wo_sb, mg, src_r, src_c0,
                              g_static=None):
                # two passes of 2 output m-tiles each: same matmul count,
                # half the PSUM banks (leaves room for the attention's
                # denominator machinery)
                if g_static is not None:
                    g = g_static
                else:
                    g = osb.tile([128, KC, TT], BF16, tag="g", bufs=2)
                    # split the 4MB load across both HWDGE rings
                    for q4 in range(4):
                        eng = nc.sync if q4 % 2 == 0 else nc.scalar
                        eng.dma_start(
                            out=g[:, q4 * 8:(q4 + 1) * 8, :],
                            in_=src_r[:, q4 * 8:(q4 + 1) * 8,
                                      src_c0:src_c0 + TT])
                for m in range(4):
                    # 32 consecutive same-bank accumulations per output
                    # tile; bufs=2 suffices (drain copy is ~0.4us vs 6.8us
                    # of accumulation) and frees a PSUM bank for den
                    omt = ops.tile([128, QO], F32, tag="om", bufs=2,
                                   name="omt")
                    for kc in range(KC):
                        nc.tensor.matmul(
                            omt,
                            lhsT=g[:, kc, m * 128:(m + 1) * 128],
                            rhs=wo_sb[:, kc, :],
                            start=kc == 0, stop=kc == KC - 1)
                    ofin = osb.tile([128, QO], F32, tag="ofin", bufs=2)
                    # drain on the Activation engine (idle during o_proj),
                    # keeping DVE free for the attention-side mask/add
                    # chains (measured: DVE here costs +62us/iter)
                    nc.scalar.copy(ofin, omt)
                    nc.gpsimd.dma_start(
                        out=out[mg * TT + m * 128:
                                mg * TT + (m + 1) * 128, :],
                        in_=ofin)

            no_collective = sim or (loop_k[1] > 1)
            with tc.tile_pool(name="fu_ps", bufs=1, space="PSUM") as cps, \
                 tc.tile_pool(name="at_sb", bufs=2) as asb, \
                 tc.tile_pool(name="wo", bufs=1) as wop, \
                 tc.tile_pool(name="o_ps", bufs=1, space="PSUM") as ops, \
                 tc.tile_pool(name="o_sb", bufs=3) as osb:
                wo_sb = wop.tile([128, KC, QO], BF16)
                nc.sync.dma_start(out=wo_sb,
                                  in_=wo_t.rearrange("(c p) m -> p c m", p=128))
                g_static = None
                if "nogather" in ablate:
                    g_static = wop.tile([128, KC, TT], BF16)
                    nc.vector.memset(g_static, 0.001)
                blk_out_r = [
                    cc_out[blk][:].rearrange("(c p) t -> p c t", p=128)
                    for blk in range(NTT)
                ]
                with phase_loop(tc):
                  # sequential block order (measured best: a reorder
                  # ending on the two small t=0 blocks left both trailing
                  # o_proj gather chains underfed, +8us)
                  prev = None
                  for blk in list(range(NTT)) + [None]:
                    if blk is not None:
                        b, t = divmod(blk, S // TT)
                        pending = None
                        for h in range(4):
                            fin = emit_attn_tile(
                                cps, asb, b, h, t,
                                cc_in[blk][h * 128:(h + 1) * 128, :],
                                sc_bufs=3, ctx_bufs=2, den_bufs=1,
                                insert_cb=pending)
                            pending = fin
                        # head 3's chain must precede the gather emission
                        # (the gather's dependency on cc writes is tracked
                        # by emission order)
                        pending()
                        if "nogather" in ablate:
                            pass
                        elif no_collective:
                            nc.sync.dma_start(out=cc_out[blk][0:QO, :],
                                              in_=cc_in[blk][:])
                        else:
                            nc.gpsimd.collective_compute(
                                "AllGather", mybir.AluOpType.bypass,
                                replica_groups=[list(range(N_CORES))],
                                ins=[cc_in[blk][:].opt()],
                                outs=[cc_out[blk][:].opt()])
                    if prev is not None and "nooproj" not in ablate:
                        emit_oproj_mg(ops, osb, wo_sb, prev,
                                      blk_out_r[prev], 0,
                                      g_static=g_static)
                    prev = blk
                if loop_k[2] > 1:
                    # empty third phase: measures pure For_i overhead
                    # (all-engine barrier + back edge) per iteration
                    dummy = osb.tile([128, 1], F32, tag="dummy")
                    with phase_loop(tc):
                        nc.vector.memset(dummy, 0.0)
    nc.compile()
    return nc


_NC_CACHE = None


def _get_nc():
    global _NC_CACHE
    if _NC_CACHE is None:
        _NC_CACHE = _build()
    return _NC_CACHE


def make_in_maps(hidden_states, position_ids, Wq, Wk, Wv, Wo):
    hs = np.ascontiguousarray(
        np.asarray(hidden_states, dtype=np.float32).reshape(TOK, HID).T
    ).astype(NPBF16)
    pos = np.asarray(position_ids, dtype=np.float32)
    inv = 1.0 / (10000.0 ** (np.arange(0, D, 2, dtype=np.float32) / D))
    fr = pos[:, None] * inv[None, :]                     # [S, 64]
    emb = np.concatenate([fr, fr], axis=-1)              # [S, D]
    cos = np.cos(emb).T.astype(np.float32)               # [D, S]
    sin = np.sin(emb).T.astype(np.float32)
    sin[:64] *= -1.0                                     # fold rotate-half sign
    perm = np.zeros((128, 128), np.float32)
    perm[np.arange(128), (np.arange(128) + 64) % 128] = 1.0
    ident = np.eye(128, dtype=np.float32)
    tri = (np.arange(128)[:, None] <= np.arange(128)[None, :]).astype(np.float32)
    ones4 = np.zeros((128, 128), np.float32)
    ones4[[0, 32, 64, 96], :] = 1.0

    scale = 1.0 / np.sqrt(D)
    Wq = np.asarray(Wq, dtype=np.float32)
    Wk = np.asarray(Wk, dtype=np.float32)
    Wv = np.asarray(Wv, dtype=np.float32)
    Wo = np.asarray(Wo, dtype=np.float32)

    in_maps = []
    for c in range(N_CORES):
        in_maps.append({
            "hid_t": hs,
            "wq_t": np.ascontiguousarray(
                (Wq[c * QO:(c + 1) * QO] * scale).T).astype(NPBF16),
            "wk_t": np.ascontiguousarray(Wk[c * D:(c + 1) * D].T).astype(NPBF16),
            "wv_t": np.ascontiguousarray(Wv[c * D:(c + 1) * D].T).astype(NPBF16),
            "wo_t": np.ascontiguousarray(Wo[c * QO:(c + 1) * QO].T).astype(NPBF16),
            "cos_t": cos,
            "sin_t": sin,
            "perm": perm.astype(NPBF16),
            "ident": ident.astype(NPBF16),
            "tri": tri.astype(NPBF16),
            "ones4": ones4.astype(NPBF16),
        })
    return in_maps


def assemble(results):
    full = np.empty((TOK, HID), np.float32)
    for c in range(N_CORES):
        full[:, c * QO:(c + 1) * QO] = results[c]["out"]
    return full.reshape(B, S, HID)


_RUNNER_CACHE = None


def _make_runner(nc):
    """Build the sharded PJRT callable once so repeat kernel() calls skip
    re-tracing; mirrors concourse.bass2jax.run_bass_via_pjrt."""
    import jax
    from jax.sharding import Mesh, PartitionSpec, NamedSharding
    from jax.experimental.shard_map import shard_map
    from concourse import bass2jax

    bass2jax.install_neuronx_cc_hook()
    partition_name = nc.partition_id_tensor.name if nc.partition_id_tensor else None
    in_names, out_names, out_avals = [], [], []
    for alloc in nc.m.functions[0].allocations:
        if not isinstance(alloc, mybir.MemoryLocationSet):
            continue
        name = alloc.memorylocations[0].name
        if alloc.kind == "ExternalInput":
            if name != partition_name:
                in_names.append(name)
        elif alloc.kind == "ExternalOutput":
            out_names.append(name)
            out_avals.append(jax.core.ShapedArray(
                tuple(alloc.tensor_shape), mybir.dt.np(alloc.dtype)))
    n_params, n_outs = len(in_names), len(out_avals)

    def _body(*args):
        operands = list(args)
        if partition_name is not None:
            operands.append(bass2jax.partition_id_tensor())
        return tuple(bass2jax._bass_exec_p.bind(
            *operands,
            out_avals=tuple(out_avals),
            in_names=tuple(in_names + out_names
                           + ([partition_name] if partition_name else [])),
            out_names=tuple(out_names),
            lowering_input_output_aliases=(),
            sim_require_finite=True,
            sim_require_nnan=True,
            nc=nc,
        ))

    devices = jax.devices()[:N_CORES]
    mesh = Mesh(np.asarray(devices), ("core",))
    fn = jax.jit(
        shard_map(_body, mesh=mesh,
                  in_specs=(PartitionSpec("core"),) * (n_params + n_outs),
                  out_specs=(PartitionSpec("core"),) * n_outs,
                  check_rep=False),
        keep_unused=True,
    )
    sharding = NamedSharding(mesh, PartitionSpec("core"))

    def run(in_maps):
        per_core = [[np.asarray(m[name]) for name in in_names] for m in in_maps]
        concat_in = [
            np.concatenate([per_core[c][i] for c in range(N_CORES)], axis=0)
            for i in range(n_params)
        ]
        concat_zeros = [
            np.zeros((N_CORES * a.shape[0], *a.shape[1:]), a.dtype)
            for a in out_avals
        ]
        import jax as _jax
        dev_args = [_jax.device_put(a, sharding)
                    for a in concat_in + concat_zeros]
        outs = fn(*dev_args)
        _jax.block_until_ready(outs)
        return [
            {name: np.asarray(outs[i]).reshape(N_CORES, *out_avals[i].shape)[c]
             for i, name in enumerate(out_names)}
            for c in range(N_CORES)
        ]

    return run


def kernel(hidden_states, position_ids, Wq, Wk, Wv, Wo):
    global _RUNNER_CACHE
    nc = _get_nc()
    in_maps = make_in_maps(hidden_states, position_ids, Wq, Wk, Wv, Wo)
    try:
        if _RUNNER_CACHE is None:
            _RUNNER_CACHE = _make_runner(nc)
        return assemble(_RUNNER_CACHE(in_maps))
    except Exception:
        from concourse.bass_utils import run_bass_kernel_spmd
        res = run_bass_kernel_spmd(nc, in_maps, core_ids=list(range(N_CORES)))
        return assemble(res.results)

